# revision 3
# baseline (speedup 1.0000x reference)
"""Trainium2 Bass kernel for a single-head BERT attention (B=8, S=2048, E=1024, H=64).

Sharding: data-parallel over batch — one batch element per NeuronCore (8 cores).
Weights replicated. No collectives.

Per-core layout (all matmuls bf16 with fp32 PSUM accumulation):
  qkT  = [Wq|Wk]^T-projection: out [128, S]  (rows 0-63 = q^T, 64-127 = k^T)
  v    = natural [S, H] per 128-row tile, with an appended ones column so the
         second matmul produces the softmax denominator for free
  wT   = scoresT[t, s] = k^T.T @ q^T  (contract over H=64)
  pT   = exp(wT / 8) (ScalarE), multiplied by the (host-transposed) mask only on
         blocks that are not all-ones; all-zero blocks are skipped entirely
  ctx  = [s, h|denom] = pT.T @ [v|1], then rows normalized by 1/denom

The program is specialized at runtime from the actual mask contents (block map of
all-zero / all-one / mixed 128x128 blocks, reduced over the batch), so any mask is
handled correctly; for the causal mask this halves the score/exp/context work.
"""

import numpy as np
import ml_dtypes

import concourse.bass as bass  # noqa: F401  (import registers bass machinery)
import concourse.bacc as bacc
import concourse.mybir as mybir
import concourse.tile as tile
from concourse.bass_utils import run_bass_kernel_spmd

BF16 = ml_dtypes.bfloat16
B, S, E, H = 8, 2048, 1024, 64
P = 128          # partitions / tile edge
NS = S // P      # 16 seq tiles
NE = E // P      # 8 embed chunks
SB = 512         # mm1 s-block (one fp32 PSUM bank)
NSB = S // SB

_cache: dict = {}
last_results = None  # BassKernelResults of the most recent run (for test harness)


def _plan_from_mask(mask: np.ndarray):
    """Derive the static block plan from the actual mask input.

    Returns (ranges, mask_blocks, mm2_lists, maskT) where the plan is valid for
    every batch element simultaneously (classifications are reduced over batch).
    """
    m = np.asarray(mask, dtype=bool)
    mt = np.ascontiguousarray(m.transpose(0, 2, 1))  # [B, t, s]
    blocks = mt.reshape(B, NS, P, NS, P)
    any_ = blocks.any(axis=(2, 4))   # [B, tj, si]
    all_ = blocks.all(axis=(2, 4))
    nz = any_.any(axis=0)            # not all-zero in some batch -> must compute
    allone = all_.all(axis=0)        # all-ones in every batch -> no mask needed
    mixed = nz & ~allone

    ranges = []
    for j in range(NS):
        cols = np.nonzero(nz[j])[0]
        if len(cols) == 0:
            ranges.append(None)
            continue
        lo = int(cols.min()) * P // SB * SB
        hi = -(-(int(cols.max()) + 1) * P // SB) * SB
        ranges.append((lo, hi))

    mask_blocks = []
    for j in range(NS):
        if ranges[j] is None:
            continue
        lo, hi = ranges[j]
        for b in range(lo // SB, hi // SB):
            if mixed[j, b * SB // P:(b + 1) * SB // P].any():
                mask_blocks.append((j, b))

    mm2 = [tuple(int(j) for j in np.nonzero(nz[:, i])[0]) for i in range(NS)]
    return ranges, mask_blocks, mm2, mt


def _build_nc(ranges, mask_blocks, mm2):
    dt = mybir.dt
    n_mb = max(len(mask_blocks), 1)
    nc = bacc.Bacc("TRN2", target_bir_lowering=False, debug=False, num_devices=8)

    xT_d = nc.dram_tensor("xT", [E, S], dt.bfloat16, kind="ExternalInput").ap()
    wqk_d = nc.dram_tensor("wqk", [E, 2 * H], dt.bfloat16, kind="ExternalInput").ap()
    wv_d = nc.dram_tensor("wv", [E, H], dt.bfloat16, kind="ExternalInput").ap()
    bqk_d = nc.dram_tensor("bqk", [1, 2 * H], dt.bfloat16, kind="ExternalInput").ap()
    bv_d = nc.dram_tensor("bv", [1, H], dt.bfloat16, kind="ExternalInput").ap()
    mb_d = nc.dram_tensor("maskb", [n_mb, P, SB], dt.bfloat16, kind="ExternalInput").ap()
    y_d = nc.dram_tensor("y", [S, H], dt.float32, kind="ExternalOutput").ap()

    EXP = mybir.ActivationFunctionType.Exp
    with tile.TileContext(nc) as tc:
        with (
            tc.tile_pool(name="consts", bufs=1) as cpool,
            tc.tile_pool(name="xt", bufs=1) as xpool,
            tc.tile_pool(name="qk", bufs=1) as qkpool,
            tc.tile_pool(name="vex", bufs=1) as vpool,
            tc.tile_pool(name="pt", bufs=1) as ppool,
            tc.tile_pool(name="maskp", bufs=1) as mpool,
            tc.tile_pool(name="outs", bufs=4) as opool,
            tc.tile_pool(name="wps", bufs=3, space="PSUM") as wpsum,
            tc.tile_pool(name="vps", bufs=2, space="PSUM") as vpsum,
            tc.tile_pool(name="cps", bufs=2, space="PSUM") as cpsum,
        ):
            # ---- constants ----
            wqk_sb = cpool.tile([P, NE, 2 * H], dt.bfloat16)
            nc.sync.dma_start(wqk_sb[:], wqk_d.rearrange("(c p) h -> p c h", p=P))
            wv_sb = cpool.tile([P, NE, H], dt.bfloat16)
            nc.sync.dma_start(wv_sb[:], wv_d.rearrange("(c p) h -> p c h", p=P))
            bqk_sb = cpool.tile([1, 2 * H], dt.bfloat16)
            nc.sync.dma_start(bqk_sb[:], bqk_d[:])
            bv_sb = cpool.tile([1, H], dt.bfloat16)
            nc.sync.dma_start(bv_sb[:], bv_d[:])
            ones_sb = cpool.tile([1, SB], dt.bfloat16)
            nc.vector.memset(ones_sb[:], 1.0)

            # ---- x^T tiles (one per 128-row embed chunk) ----
            xt = []
            for c in range(NE):
                t = xpool.tile([P, S], dt.bfloat16, tag=f"xt{c}", name=f"xt{c}")
                nc.sync.dma_start(t[:], xT_d[c * P:(c + 1) * P, :])
                xt.append(t)

            # ---- mask blocks ----
            mask_tiles = {}
            for idx, (j, b) in enumerate(mask_blocks):
                mt_sb = mpool.tile([P, SB], dt.bfloat16, tag=f"mb{idx}", name=f"mb{idx}")
                nc.sync.dma_start(mt_sb[:], mb_d[idx])
                mask_tiles[(j, b)] = mt_sb

            # ---- q/k projection: qkT[0:64] = q^T, qkT[64:128] = k^T ----
            qkT_sb = qkpool.tile([P, S], dt.bfloat16)
            kT_sb = qkpool.tile([64, S], dt.bfloat16)
            for sb_i in range(NSB):
                ps = wpsum.tile([P, SB], dt.float32, tag="wps", name="wps")
                for c in range(NE):
                    nc.tensor.matmul(
                        ps[:], wqk_sb[:, c, :], xt[c][:, sb_i * SB:(sb_i + 1) * SB],
                        start=(c == 0), stop=False)
                nc.tensor.matmul(ps[:], bqk_sb[:], ones_sb[:], start=False, stop=True)
                nc.vector.tensor_copy(qkT_sb[:, sb_i * SB:(sb_i + 1) * SB], ps[:])
            # partition fixup: k^T down to partitions 0-63
            nc.sync.dma_start(kT_sb[:], qkT_sb[64:128, :])

            # ---- v (natural) per seq tile, with ones column ----
            vext = []
            for j in range(NS):
                vt = vpool.tile([P, H + 1], dt.bfloat16, tag=f"vx{j}", name=f"vx{j}")
                nc.vector.memset(vt[:, H:H + 1], 1.0)
                pv = vpsum.tile([P, H], dt.float32, tag="vps", name="vps")
                for c in range(NE):
                    nc.tensor.matmul(
                        pv[:], xt[c][:, j * P:(j + 1) * P], wv_sb[:, c, :],
                        start=(c == 0), stop=False)
                nc.tensor.matmul(pv[:], ones_sb[:, 0:P], bv_sb[:], start=False, stop=True)
                nc.vector.tensor_copy(vt[:, 0:H], pv[:])
                vext.append(vt)

            # ---- scoresT -> exp -> mask ----
            pt = [ppool.tile([P, S], dt.bfloat16, tag=f"pt{j}", name=f"pt{j}") for j in range(NS)]
            for j in range(NS):
                if ranges[j] is None:
                    continue
                lo, hi = ranges[j]
                for b in range(lo // SB, hi // SB):
                    ps = wpsum.tile([P, SB], dt.float32, tag="wps", name="wps")
                    nc.tensor.matmul(
                        ps[:], kT_sb[:, j * P:(j + 1) * P],
                        qkT_sb[0:64, b * SB:(b + 1) * SB], start=True, stop=True)
                    sl = pt[j][:, b * SB:(b + 1) * SB]
                    nc.scalar.activation(sl, ps[:], EXP, scale=0.125)
                    if (j, b) in mask_tiles:
                        nc.vector.tensor_mul(sl, sl, mask_tiles[(j, b)][:])

            # ---- context + denominator + normalize ----
            for i in range(NS):
                js = mm2[i]
                ob = opool.tile([P, H], dt.float32, tag="out", name="ob")
                if not js:
                    nc.vector.memset(ob[:], 0.0)
                else:
                    pc = cpsum.tile([P, H + 1], dt.float32, tag="cps", name="pc")
                    for n, j in enumerate(js):
                        nc.tensor.matmul(
                            pc[:], pt[j][:, i * P:(i + 1) * P], vext[j][:],
                            start=(n == 0), stop=(n == len(js) - 1))
                    rc = opool.tile([P, 1], dt.float32, tag="recip", name="rc")
                    nc.vector.reciprocal(rc[:], pc[:, H:H + 1])
                    nc.vector.tensor_scalar_mul(ob[:], pc[:, 0:H], rc[:])
                nc.sync.dma_start(y_d[i * P:(i + 1) * P, :], ob[:])

    nc.compile()
    return nc


def kernel(x, mask, Wq, bq, Wk, bk, Wv, bv, _trace=False, _trace_kwargs=None):
    global last_results
    x = np.asarray(x, dtype=np.float32)
    ranges, mask_blocks, mm2, maskT = _plan_from_mask(mask)

    key = (tuple(ranges), tuple(mask_blocks), tuple(mm2))
    nc = _cache.get(key)
    if nc is None:
        nc = _build_nc(ranges, mask_blocks, mm2)
        _cache[key] = nc

    wqk = np.concatenate([np.asarray(Wq), np.asarray(Wk)], axis=1).astype(BF16)
    wv = np.asarray(Wv).astype(BF16)
    bqk = np.concatenate([np.asarray(bq), np.asarray(bk)])[None, :].astype(BF16)
    bvv = np.asarray(bv)[None, :].astype(BF16)

    in_maps = []
    for b in range(B):
        xT_b = np.ascontiguousarray(x[b].T).astype(BF16)
        if mask_blocks:
            mb = np.stack([
                maskT[b, j * P:(j + 1) * P, blk * SB:(blk + 1) * SB]
                for (j, blk) in mask_blocks]).astype(BF16)
        else:
            mb = np.zeros((1, P, SB), dtype=BF16)
        in_maps.append({
            "xT": xT_b, "wqk": wqk, "wv": wv, "bqk": bqk, "bv": bvv, "maskb": mb,
        })

    res = run_bass_kernel_spmd(
        nc, in_maps, core_ids=list(range(B)),
        trace=_trace, **(_trace_kwargs or {}))
    last_results = res
    return np.stack([res.results[b]["y"] for b in range(B)])
